# revision 33
# baseline (speedup 1.0000x reference)
"""Trainium2 Bass kernel for a cross-attention block.

Problem (hardcoded shapes): B=4, S=2048, T=256, H=2048, NH=16, HD=128.
  q = hs @ Wq.T + bq ; k = at @ Wk.T + bk ; v = at @ Wv.T + bv   (per-head 128-dim)
  scores = q k^T / sqrt(128), masked over audio positions, softmax over T
  out = LN(clip(rs) * (ctx @ Wo.T + bo)) * gamma + beta

Sharding: pure data parallel over (batch, S/2) -> 8 shards, one per NeuronCore.

All streamed operands are host-prepacked into consumption-ordered slabs whose
per-partition lines are >=2KB contiguous (plain 3D copies, no strided DMA
gathers), and every input DMA rides the sync ring in exact consumption order
(ring order == priority, so no dep-gating is needed):
  at -> wk (K proj) -> hs0/wq0 ... wq3, hs1 (Q proj) -> wv (V proj)
  -> wo e-half0 stream + wo e-half1 resident (out proj)
Phase order is K -> Q -> V so the DMA-hungry wv stream rides under Q proj's
huge DMA slack instead of crunching at kernel start.  Output, and the delta
spill used by LayerNorm, are bf16 (tolerance is 2e-2; bf16 adds ~2.5e-3):
the spill fits both ping-pong groups in the dead hs slot, freeing the dead
qT slot to hold Wo's e-half1 resident (cuts Wo traffic 16.8MB -> 12.6MB and
lets the last group's final pass split per s-tile for a ~3us exit tail).

Device pipeline (zero on-device transposes):
  - K^T  [d, t]  = sum_c WkT[c, d-tile] x atT[c, t]         (lhsT=WkT, rhs=atT)
  - Q^T  [d, s]  = sum_c WqT[c, d-tile] x hsT[c, s]         (lhsT=WqT, rhs=hsT)
  - V    [t, d]  = sum_c atT[c, t-tile] x WvT[c, d]         (lhsT=atT, rhs=WvT)
  - S^T  [t, s]  = K^T-tile.T x Q^T   per head              (lhsT=K^T,  rhs=Q^T)
  - expT [t, s]  = Exp(scale * S^T + mask_bias[t])          (ACT, mask as bias)
  - den  [128,s] = ones[t,128].T x expT  (PE column-sum, bcast on partitions)
  - ctxT [d, s]  = V-tile.T x expT, ctxT *= 1/den (DVE recip + mul)
  - delta[s, e]  = sum_c ctxT[c, s-tile].T x WoT[c, e]      (lhsT=ctxT, rhs=WoT)
  - LN over e (free axis) via bn_stats/bn_aggr; residual_scale folded into rstd:
      out = (delta - mu) / sqrt(var + eps/rs^2)   [* gamma + beta if nontrivial]
"""

import math
import os
import sys

import numpy as np

for _p in ("/opt/trn_rl_repo", "/root/.axon_site/_ro/trn_rl_repo"):
    if os.path.isdir(_p) and _p not in sys.path:
        sys.path.insert(0, _p)

import ml_dtypes

import concourse.bass as bass
import concourse.mybir as mybir
import concourse.tile as tile
from concourse import bacc

BF16 = mybir.dt.bfloat16
F32 = mybir.dt.float32
AF = mybir.ActivationFunctionType
ALU = mybir.AluOpType

P = 128
MASK_NEG = -100.0  # additive bias for masked keys; exp(-100+~6) == 0 in fp32
EPS_LN = 1e-5
RES_SCALE_MAX = 0.3


def _bcast_row_ap(ap_1d, rows):
    """DRAM [N] -> broadcast AP [rows, N] (partition stride 0)."""
    return bass.AP(tensor=ap_1d.tensor, offset=ap_1d.offset,
                   ap=[[0, rows], list(ap_1d.ap[0])])


def emit_cross_attn(tc, io, S, T, H, NH, rs, use_qkv_bias, use_gamma_beta,
                    dedup=False):
    """Emit the full per-core pipeline. io maps name -> DRAM AP.

    dedup: each core of a (batch, s-half) pair projects only half of K (by
    head) and half of V (by output column) from host-halved weight slabs,
    then the pair AllGathers the halves (1MB each way, hidden under Q proj).
    Halves both the K/V projection PE time and the early-phase weight DMA.
    """
    nc = tc.nc
    from concourse.bass import _add_dep_helper
    C = H // P            # contraction chunks (== NH when HD==128)
    CG = 4                # c-chunks per streamed weight DMA
    NCG = C // CG
    NT = T // P           # t chunks
    SB = min(512, S)      # s block (matmul moving free dim)
    NSB = S // SB         # s blocks
    NST = S // P          # s tiles
    EB = min(512, H)      # free-dim block for weight streaming
    NEB = H // EB
    DG = EB // P          # d tiles per streamed weight block
    NDB = C // DG         # d blocks
    EW = (H // 2) if NEB >= 2 else H   # e-half width for the out proj
    NEBP = 2 if NEB >= 2 else 1
    EBL = NEB // NEBP
    scale = 1.0 / max(math.sqrt(128.0), 1e-8)

    atp, hsp = io["atp"], io["hsp"]
    wkp, wqp, wvp, wop = io["wkp"], io["wqp"], io["wvp"], io["wop"]
    maskb, out = io["maskb"], io["out"]

    _ld_ctr = [0]

    def ld_eng():
        _ld_ctr[0] += 1
        return nc.sync

    # the bias/gamma variants carry extra fp32 residents; shrink the
    # streaming/scratch buffer counts there to stay inside SBUF (those
    # variants are compile-complete but not perf-tuned)
    tight = use_qkv_bias or use_gamma_beta
    WST_BUFS = 2 if tight else 4
    WO_BUFS = 3 if tight else 5
    EXP_BUFS = 4 if tight else 8
    BC_BUFS = 1 if tight else 2
    O_BUFS = 2 if tight else 4

    # ---- pools ----
    with (
        tc.tile_pool(name="big", bufs=1) as big,     # WqT slot, reused by ctxT
        tc.tile_pool(name="res", bufs=1) as res,     # resident activations
        tc.tile_pool(name="wst", bufs=WST_BUFS) as wst,     # streamed Wk/Wv/Wo chunks
        tc.tile_pool(name="sml", bufs=4) as sml,     # exp/LN intermediates
        tc.tile_pool(name="psum", bufs=8, space="PSUM") as psum,
    ):
        ones_mat = res.tile([P, P], BF16, tag="ones_mat")
        nc.vector.memset(ones_mat, 1.0)
        inv_rs2 = (EPS_LN / (rs * rs)) if rs > 0 else 3.0e38
        eps_sb = res.tile([P, 1], F32, tag="eps")
        nc.vector.memset(eps_sb, inv_rs2)

        if use_qkv_bias:
            bq_sb = res.tile([P, C], F32, tag="bq")
            nc.gpsimd.dma_start(bq_sb, io["bq"].rearrange("(c p) -> p c", p=P))
            bk_sb = res.tile([P, C], F32, tag="bk")
            nc.gpsimd.dma_start(bk_sb, io["bk"].rearrange("(c p) -> p c", p=P))
            bv_sb = res.tile([P, H], F32, tag="bv")
            nc.gpsimd.dma_start(bv_sb, _bcast_row_ap(io["bv"], P))
            bo_sb = res.tile([P, H], F32, tag="bo")
            nc.gpsimd.dma_start(bo_sb, _bcast_row_ap(io["bo"], P))
        if use_gamma_beta:
            gm_sb = res.tile([P, H], BF16, tag="gamma")
            nc.gpsimd.dma_start(gm_sb, _bcast_row_ap(io["gamma"], P))
            bt_sb = res.tile([P, H], BF16, tag="beta")
            nc.gpsimd.dma_start(bt_sb, _bcast_row_ap(io["beta"], P))

        # ---- K projection: kT_sb[p, h, t] = K^T (d on partitions) ----
        # Ring: at chunk cg + wk dg0 chunk cg interleaved, then wk dg1..
        # With dedup, each core computes only its half of the d range (the
        # host supplies the matching half of the wk slabs).
        NKD = NDB // 2 if dedup else NDB
        NVE = NEB // 2 if dedup else NEB
        at_sb = res.tile([P, C, T], BF16, tag="at")
        kT_sb = res.tile([P, C, T], BF16, tag="kT")
        for dg in range(NKD):
            kps = [psum.tile([P, T], F32, tag="ps", name=f"kps{dl}")
                   for dl in range(DG)]
            for cg in range(NCG):
                cs = slice(cg * CG, (cg + 1) * CG)
                wk_g = wst.tile([P, CG, EB], BF16, tag="wkv", name="wk_g")
                if dg == 0 and cg == 0:
                    # sub-chunked so the very first matmuls start sooner
                    for h0 in range(2):
                        hs_ = slice(h0 * CG // 2, (h0 + 1) * CG // 2)
                        ld_eng().dma_start(at_sb[:, hs_, :], atp[:, hs_, :])
                        ld_eng().dma_start(wk_g[:, hs_, :],
                                           wkp[dg, :, hs_, :])
                elif dg == 0:
                    ld_eng().dma_start(at_sb[:, cs, :], atp[:, cs, :])
                    ld_eng().dma_start(wk_g, wkp[dg, :, cs, :])
                else:
                    ld_eng().dma_start(wk_g, wkp[dg, :, cs, :])
                for cl in range(CG):
                    c = cg * CG + cl
                    for dl in range(DG):
                        nc.tensor.matmul(
                            kps[dl], wk_g[:, cl, dl * P:(dl + 1) * P],
                            at_sb[:, c, :],
                            start=(c == 0), stop=(c == C - 1),
                        )
            for dl in range(DG):
                d = dg * DG + dl
                if use_qkv_bias:
                    nc.scalar.activation(kT_sb[:, d, :], kps[dl], AF.Identity,
                                         bias=bk_sb[:, d:d + 1])
                else:
                    nc.scalar.copy(kT_sb[:, d, :], kps[dl])

        # ---- V projection: v_sb[p, tt, d] = V (t on partitions) ----
        # Without dedup it is emitted after Q proj so the wv stream rides
        # under Q proj's DMA slack; with dedup each core computes only its
        # e-half (half the PE and half the DMA) right after K, and the
        # halves are exchanged below while Q proj runs.
        v_sb = res.tile([P, NT, H], BF16, tag="v")

        def emit_v_proj(eb_list):
            for eb in eb_list:
                vps = [psum.tile([P, EB], F32, tag="ps", name=f"vps{tt}")
                       for tt in range(NT)]
                for cg in range(NCG):
                    cs = slice(cg * CG, (cg + 1) * CG)
                    wv_g = wst.tile([P, CG, EB], BF16, tag="wkv", name="wv_g")
                    ld_eng().dma_start(wv_g, wvp[eb, :, cs, :])
                    for cl in range(CG):
                        c = cg * CG + cl
                        for tt in range(NT):
                            nc.tensor.matmul(
                                vps[tt], at_sb[:, c, tt * P:(tt + 1) * P],
                                wv_g[:, cl, :],
                                start=(c == 0), stop=(c == C - 1),
                            )
                for tt in range(NT):
                    nc.scalar.copy(v_sb[:, tt, eb * EB:(eb + 1) * EB],
                                   vps[tt])
            if use_qkv_bias:
                for tt in range(NT):
                    nc.vector.tensor_add(v_sb[:, tt, :], v_sb[:, tt, :],
                                         bv_sb)

        def emit_collective():
            # ---- pair AllGather of the K/V halves (rides TOPSP/SDMA
            # silicon + the gpsimd ring; hidden under Q proj) ----
            KHW = (C // 2) * T
            VHW = NT * (H // 2)
            cc_in, cc_out = io["cc_in"], io["cc_out"]
            b1 = nc.gpsimd.dma_start(
                cc_in[:, 0:KHW],
                kT_sb[:, 0:C // 2, :].rearrange("p c t -> p (c t)"))
            vbs = [nc.gpsimd.dma_start(
                cc_in[:, KHW + tt * (H // 2):KHW + (tt + 1) * (H // 2)],
                v_sb[:, tt, 0:H // 2]) for tt in range(NT)]
            cc = nc.gpsimd.collective_compute(
                "AllGather", mybir.AluOpType.bypass,
                replica_groups=[[2 * i, 2 * i + 1] for i in range(4)],
                ins=[cc_in[:]], outs=[cc_out[:]])
            for b in [b1] + vbs:
                _add_dep_helper(cc.ins, b.ins, sync=True,
                                reason="collective reads bounce buffer")
            rbs = [
                nc.gpsimd.dma_start(
                    kT_sb[:, 0:C // 2, :].rearrange("p c t -> p (c t)"),
                    cc_out[0, :, 0:KHW]),
                nc.gpsimd.dma_start(
                    kT_sb[:, C // 2:C, :].rearrange("p c t -> p (c t)"),
                    cc_out[1, :, 0:KHW]),
            ]
            for tt in range(NT):
                lo, hi = KHW + tt * (H // 2), KHW + (tt + 1) * (H // 2)
                rbs.append(nc.gpsimd.dma_start(
                    v_sb[:, tt, 0:H // 2], cc_out[0, :, lo:hi]))
                rbs.append(nc.gpsimd.dma_start(
                    v_sb[:, tt, H // 2:H], cc_out[1, :, lo:hi]))
            for rb in rbs:
                _add_dep_helper(rb.ins, cc.ins, sync=True,
                                reason="readback after gather")

        # ---- Q projection: qT_sb[p, h, s] = Q^T (WqT resident) ----
        # Ring: hs sb0 / wq db0 interleaved (Q's first matmuls), wq db1..3,
        # hs sb1.  No pacing gates: ring order is the priority.
        hs_sb = res.tile([P, NSB, C, SB], BF16, tag="hsT")
        wq_sb = big.tile([P, NDB, C, EB], BF16, tag="big")
        qT_sb = res.tile([P, C, S], BF16, tag="qT")

        def q_dma_first():
            for cg in range(NCG):
                cs = slice(cg * CG, (cg + 1) * CG)
                ld_eng().dma_start(hs_sb[:, 0, cs, :], hsp[0, :, cs, :])
                ld_eng().dma_start(wq_sb[:, 0, cs, :], wqp[0, :, cs, :])

        def q_dma_rest():
            for db in range(1, NDB):
                for cg in range(NCG):
                    cs = slice(cg * CG, (cg + 1) * CG)
                    ld_eng().dma_start(wq_sb[:, db, cs, :], wqp[db, :, cs, :])
            for sb in range(1, NSB):
                for cg in range(NCG):
                    cs = slice(cg * CG, (cg + 1) * CG)
                    ld_eng().dma_start(hs_sb[:, sb, cs, :], hsp[sb, :, cs, :])

        def q_mm(sb, d):
            ps = psum.tile([P, SB], F32, tag="ps", name="qps")
            for c in range(C):
                nc.tensor.matmul(
                    ps, wq_sb[:, d // DG, c, (d % DG) * P:(d % DG + 1) * P],
                    hs_sb[:, sb, c, :],
                    start=(c == 0), stop=(c == C - 1),
                )
            if use_qkv_bias:
                nc.scalar.activation(qT_sb[:, d, sb * SB:(sb + 1) * SB],
                                     ps, AF.Identity, bias=bq_sb[:, d:d + 1])
            else:
                nc.scalar.copy(qT_sb[:, d, sb * SB:(sb + 1) * SB], ps)

        if dedup:
            # Interleave V's two e-blocks and Q's first d-group so the
            # early phase's DMA-paced dead-byte gaps (hs0/wq0 lead-in)
            # smear into sub-us dribble instead of 3-5us holes that cost
            # a HAM re-throttle on top of the idle itself.
            emit_v_proj([0])
            q_dma_first()
            for d in range(DG):
                q_mm(0, d)
            emit_v_proj(list(range(1, NVE)))
            emit_collective()
            q_dma_rest()
            for d in range(DG, C):
                q_mm(0, d)
            for sb in range(1, NSB):
                for d in range(C):
                    q_mm(sb, d)
        else:
            q_dma_first()
            q_dma_rest()
            for sb in range(NSB):
                for d in range(C):
                    q_mm(sb, d)

        mb_sb = res.tile([P, NT], F32, tag="maskb")
        nc.sync.dma_start(mb_sb, maskb)

        if not dedup:
            emit_v_proj(list(range(NVE)))

        # ---- attention -> ctxT (reuses the WqT SBUF slot) ----
        # Per head, all PE work (scores, den, ctx) is emitted contiguously so
        # the PE stream never waits on the DVE reciprocal.
        ctx_sb = big.tile([P, C, S], BF16, tag="big")

        def attn_tail(prev):
            # den/ctx (PE) + normalize (DVE) for an already-exp'd pair
            s_sl, pair, exps = prev
            for h in pair:
                # all-ones [128,128] stationary: out[p, s] = den[s] on every
                # partition; feeds a full-width fast-approx reciprocal.
                ps_den = psum.tile([P, SB], F32, tag="ps", name="den")
                for tt in range(NT):
                    nc.tensor.matmul(ps_den, ones_mat, exps[h][tt],
                                     start=(tt == 0), stop=(tt == NT - 1))
                ps_c = psum.tile([P, SB], F32, tag="ps", name="ctxT")
                for tt in range(NT):
                    nc.tensor.matmul(
                        ps_c, v_sb[:, tt, h * P:(h + 1) * P], exps[h][tt],
                        start=(tt == 0), stop=(tt == NT - 1),
                    )
                bc_sb = sml.tile([P, SB], F32, tag="bcast_sb", bufs=BC_BUFS)
                nc.vector.reciprocal_approx_fast(bc_sb, ps_den)
                nc.vector.tensor_mul(ctx_sb[:, h, s_sl], ps_c, bc_sb)

        # Software-pipelined across head pairs: pair p's scores (PE) are
        # emitted before pair p-1's den/ctx, so the PE always has exp-
        # independent work in flight while ACT computes pair p's exps.
        # PSUM: 4 score banks (pair p) + 4 den/ctx banks (pair p-1) == 8.
        prev = None
        for sb in range(NSB):
            s_sl = slice(sb * SB, (sb + 1) * SB)
            for h0 in range(0, NH, 2):
                pair = (h0, h0 + 1)
                exps = {}
                for h in pair:
                    exps[h] = []
                    for tt in range(NT):
                        ps_s = psum.tile([P, SB], F32, tag="ps",
                                         name="scoresT")
                        nc.tensor.matmul(
                            ps_s, kT_sb[:, h, tt * P:(tt + 1) * P],
                            qT_sb[:, h, s_sl], start=True, stop=True,
                        )
                        e_sb = sml.tile([P, SB], BF16, tag="expT", bufs=EXP_BUFS)
                        nc.scalar.activation(e_sb, ps_s, AF.Exp,
                                             bias=mb_sb[:, tt:tt + 1],
                                             scale=scale)
                        exps[h].append(e_sb)
                if prev is not None:
                    attn_tail(prev)
                prev = (s_sl, pair, exps)
        attn_tail(prev)

        # ---- out projection + LayerNorm, [s, e] layout ----
        # SG4 s-tiles per group x NEBP passes over disjoint e-halves.  The
        # e-half0 pass streams wo chunks; e-half1 reads a resident copy of
        # WoT[:, H/2:] parked in the dead qT slot.  Deltas spill as bf16 to
        # the dead hs slot (both ping-pong groups fit) and LayerNorm runs
        # from SBUF, split across DVE and ACT.
        SG4 = min(4, NST)
        n_sg4 = NST // SG4

        def emit_pass(sg4, ebp, si_list, d_big, stats_big, wo_res):
            dps = {(si, ebl): psum.tile([P, EB], F32, tag="ps",
                                        name=f"delta{si}_{ebl}")
                   for si in si_list for ebl in range(EBL)}
            for cp in range(C // 2):
                if wo_res is None:
                    # c-pair chunks (one DMA) on the sync ring
                    wo_t = wst.tile([P, 2, EW], BF16, tag="wo", name="wo_t",
                                    bufs=WO_BUFS)
                    nc.sync.dma_start(wo_t,
                                      wop[ebp, :, cp * 2:(cp + 1) * 2, :])
                    rd = lambda cl, ebl: wo_t[:, cl, ebl * EB:(ebl + 1) * EB]
                else:
                    rd = lambda cl, ebl: wo_res[:, cp * 2 + cl,
                                                ebl * EB:(ebl + 1) * EB]
                for cl in range(2):
                    c = cp * 2 + cl
                    for si in si_list:
                        st = sg4 * SG4 + si
                        for ebl in range(EBL):
                            nc.tensor.matmul(
                                dps[(si, ebl)],
                                ctx_sb[:, c, st * P:(st + 1) * P],
                                rd(cl, ebl),
                                start=(c == 0), stop=(c == C - 1),
                            )
            for si in si_list:
                for ebl in range(EBL):
                    eb = ebp * EBL + ebl
                    nc.scalar.copy(d_big[:, si, eb, :], dps[(si, ebl)])
                    if stats_big is not None:
                        # LN stats inline with the spill: the DVE is idle
                        # during the out-proj matmuls.
                        nc.vector.bn_stats(stats_big[:, si, eb, :],
                                           d_big[:, si, eb, :])

        def emit_ln(sg4, si, d_big, stats_big, fine=False):
            # fine: halve the normalize/write chunk size so the kernel-exit
            # write flush overlaps the normalize chain (last LN only)
            st = sg4 * SG4 + si
            if use_qkv_bias:
                nc.vector.tensor_add(
                    d_big[:, si].rearrange("p a b -> p (a b)"),
                    d_big[:, si].rearrange("p a b -> p (a b)"), bo_sb)
            if stats_big is None:
                stats = sml.tile([P, NEB, 6], F32, tag="stats")
                for eb in range(NEB):
                    nc.vector.bn_stats(stats[:, eb, :], d_big[:, si, eb, :])
            else:
                stats = stats_big[:, si]
            mv = sml.tile([P, 2], F32, tag="mv")
            nc.vector.bn_aggr(mv, stats)
            sd = sml.tile([P, 1], F32, tag="sd")
            nc.scalar.activation(sd, mv[:, 1:2], AF.Sqrt, bias=eps_sb)
            rstd = sml.tile([P, 1], F32, tag="rstd")
            nc.vector.reciprocal(rstd, sd)
            act_split = not use_gamma_beta
            if act_split:
                # -mu * rstd, so ACT can do (x*rstd + nmr) for half the ebs
                nmr = sml.tile([P, 1], F32, tag="nmr")
                nc.vector.tensor_scalar(nmr, mv[:, 0:1], rstd, -1.0,
                                        op0=ALU.mult, op1=ALU.mult)
            nch = NEB * 2 if fine else NEB
            W = H // nch
            dflat = d_big[:, si].rearrange("p a b -> p (a b)")
            for j in range(nch):
                sl = slice(j * W, (j + 1) * W)
                o_sb = sml.tile([P, W], BF16, tag="o_sb", bufs=O_BUFS)
                if act_split and j >= nch - max(nch // 4, 1):
                    nc.scalar.activation(o_sb, dflat[:, sl],
                                         AF.Identity, bias=nmr[:, 0:1],
                                         scale=rstd[:, 0:1])
                else:
                    nc.vector.tensor_scalar(
                        o_sb, dflat[:, sl], mv[:, 0:1], rstd,
                        op0=ALU.subtract, op1=ALU.mult,
                    )
                    if use_gamma_beta:
                        nc.vector.tensor_mul(o_sb, o_sb, gm_sb[:, sl])
                        nc.vector.tensor_add(o_sb, o_sb, bt_sb[:, sl])
                # writes ride the gpsimd ring (sync carries the wo stream);
                # the last group alternates so the exit tail drains 2-wide.
                if sg4 < n_sg4 - 1:
                    eng = nc.gpsimd
                else:
                    eng = nc.sync if j % 2 == 0 else nc.gpsimd
                eng.dma_start(out[st * P:(st + 1) * P, sl], o_sb)

        # both ping-pong delta groups fit in the dead hs slot as bf16
        d_all = res.tile([P, 2, SG4, NEB, EB], BF16, tag="hsT", name="d_all")
        wo_res = None
        for sg4 in range(n_sg4):
            d_big = d_all[:, sg4 % 2]
            stats_big = (None if use_qkv_bias else
                         sml.tile([P, SG4, NEB, 6], F32, tag="stats_big",
                                  bufs=2, name=f"stats{sg4}"))
            last = (sg4 == n_sg4 - 1)
            if NEBP == 1:
                emit_pass(sg4, 0, list(range(SG4)), d_big, stats_big, None)
                for si in range(SG4):
                    emit_ln(sg4, si, d_big, stats_big)
                continue
            emit_pass(sg4, 0, list(range(SG4)), d_big, stats_big, None)
            if sg4 == 0:
                # park WoT's e-half1 in the dead qT slot (ring position:
                # after this group's e-half0 chunks; the DMA's WAR on qT
                # waits out attention by itself)
                wo_res = res.tile([P, C, EW], BF16, tag="qT", name="wo_res")
                for cg in range(NCG):
                    cs = slice(cg * CG, (cg + 1) * CG)
                    nc.sync.dma_start(wo_res[:, cs, :], wop[1, :, cs, :])
            if not last or SG4 < 4:
                emit_pass(sg4, 1, list(range(SG4)), d_big, stats_big, wo_res)
                for si in range(SG4):
                    emit_ln(sg4, si, d_big, stats_big)
            else:
                # Last group: per-s-tile e-half1 passes (no extra Wo traffic,
                # it's resident), with each LN emitted immediately after its
                # own pass.  The ACT/DVE queues are FIFO, so emitting ln(i)
                # after pass(i+1) would chain ln(i)'s ops behind pass(i+1)'s
                # PSUM copies and push every LN to the very end; this order
                # lets ln(i) overlap pass(i+1)'s matmuls instead.
                for si in range(SG4):
                    emit_pass(sg4, 1, [si], d_big, stats_big, wo_res)
                    emit_ln(sg4, si, d_big, stats_big)


def build_nc(S, T, H, NH, rs, use_qkv_bias=False, use_gamma_beta=False,
             dedup=True):
    nc = bacc.Bacc("TRN2", target_bir_lowering=False, debug=False,
                   num_devices=8)
    C = H // P
    NDB = H // 512
    NSB = S // 512
    NEB = H // 512
    NKD = NDB // 2 if dedup else NDB
    NVE = NEB // 2 if dedup else NEB
    io = {
        "atp": nc.dram_tensor("atp", [P, C, T], BF16, kind="ExternalInput")[:],
        "hsp": nc.dram_tensor("hsp", [NSB, P, C, 512], BF16,
                              kind="ExternalInput")[:],
        "wkp": nc.dram_tensor("wkp", [NKD, P, C, 512], BF16,
                              kind="ExternalInput")[:],
        "wqp": nc.dram_tensor("wqp", [NDB, P, C, 512], BF16,
                              kind="ExternalInput")[:],
        "wvp": nc.dram_tensor("wvp", [NVE, P, C, 512], BF16,
                              kind="ExternalInput")[:],
        "wop": nc.dram_tensor("wop", [2, P, C, H // 2], BF16,
                              kind="ExternalInput")[:],
        "maskb": nc.dram_tensor("maskb", [P, T // P], F32,
                                kind="ExternalInput")[:],
        "out": nc.dram_tensor("out", [S, H], BF16, kind="ExternalOutput")[:],
    }
    if dedup:
        hw = (C // 2) * T + (T // P) * (H // 2)   # per-partition gather elems
        io["cc_in"] = nc.dram_tensor("cc_in", [P, hw], BF16,
                                     kind="Internal")[:]
        io["cc_out"] = nc.dram_tensor("cc_out", [2, P, hw], BF16,
                                      kind="Internal")[:]
    if use_qkv_bias:
        for n in ("bq", "bk", "bv", "bo"):
            io[n] = nc.dram_tensor(n, [H], F32, kind="ExternalInput")[:]
    if use_gamma_beta:
        for n in ("gamma", "beta"):
            io[n] = nc.dram_tensor(n, [H], BF16, kind="ExternalInput")[:]
    with tile.TileContext(nc) as tc:
        emit_cross_attn(tc, io, S, T, H, NH, rs, use_qkv_bias, use_gamma_beta,
                        dedup=dedup)
    nc.finalize()
    return nc


def _nan_clean(x, lim=10000.0):
    return np.nan_to_num(x, nan=0.0, posinf=lim, neginf=-lim)


def prepare_in_maps(hidden_states, audio_tokens, attention_mask,
                    Wq, bq, Wk, bk, Wv, bv, Wo, bo,
                    use_qkv_bias, n_cores=8):
    """Host-side shard + transpose + bf16 cast + slab packing.

    Streamed tensors are packed [slab][p][c][blk] so each DMA chunk's
    per-partition line is contiguous (>=2KB) in DRAM.
    """
    bf = ml_dtypes.bfloat16
    B, S_full, H = hidden_states.shape
    T = audio_tokens.shape[1]
    halves = n_cores // B
    Sc = S_full // halves
    C = H // P
    hs = _nan_clean(np.asarray(hidden_states, np.float32))
    at = _nan_clean(np.asarray(audio_tokens, np.float32))

    def packw(W, blk):
        WT = np.asarray(W, np.float32).T.astype(bf)        # [H(c), H(out)]
        nb = WT.shape[1] // blk
        return np.ascontiguousarray(
            WT.reshape(C, P, nb, blk).transpose(2, 1, 0, 3))

    wkp = packw(Wk, 512)        # [NDB, 128, C, 512]
    wqp = packw(Wq, 512)
    wvp = packw(Wv, 512)
    wop = packw(Wo, H // 2)     # [2, 128, C, H//2]
    dedup = not use_qkv_bias
    nkd = wkp.shape[0] // 2
    nve = wvp.shape[0] // 2
    in_maps = []
    for i in range(n_cores):
        b, half = i // halves, i % halves
        if dedup:
            # pair rank0 (even core) projects the low K-head / V-column
            # half, rank1 the high half; the kernel AllGathers them.
            wk_i = wkp[:nkd] if i % 2 == 0 else wkp[nkd:]
            wv_i = wvp[:nve] if i % 2 == 0 else wvp[nve:]
        else:
            wk_i, wv_i = wkp, wvp
        hsT = hs[b, half * Sc:(half + 1) * Sc, :].T.astype(bf)   # [H, Sc]
        hsp = np.ascontiguousarray(
            hsT.reshape(C, P, Sc // 512, 512).transpose(2, 1, 0, 3))
        atT = at[b].T.astype(bf)                                  # [H, T]
        atp = np.ascontiguousarray(atT.reshape(C, P, T).transpose(1, 0, 2))
        m = np.asarray(attention_mask[b])
        mbias = np.where(m > 0.5, 0.0, MASK_NEG).astype(np.float32)
        maskb = np.ascontiguousarray(mbias.reshape(-1, P).T)  # [P, T//P]
        im = {"atp": atp, "hsp": hsp, "wkp": wk_i, "wqp": wqp,
              "wvp": wv_i, "wop": wop, "maskb": maskb}
        if use_qkv_bias:
            im["bq"] = np.asarray(bq, np.float32)
            im["bk"] = np.asarray(bk, np.float32)
            im["bv"] = np.asarray(bv, np.float32)
            im["bo"] = np.asarray(bo, np.float32)
        in_maps.append(im)
    return in_maps


_NC_CACHE = {}


def get_nc(Sc, T, H, NH, rs, use_qkv_bias, use_gamma_beta):
    key = (Sc, T, H, NH, rs, use_qkv_bias, use_gamma_beta)
    if key not in _NC_CACHE:
        _NC_CACHE[key] = build_nc(Sc, T, H, NH, rs, use_qkv_bias,
                                  use_gamma_beta, dedup=not use_qkv_bias)
    return _NC_CACHE[key]


def kernel(hidden_states, audio_tokens, attention_mask,
           Wq, bq, Wk, bk, Wv, bv, Wo, bo, ln_gamma, ln_beta, residual_scale):
    from concourse.bass_utils import run_bass_kernel_spmd

    B, S_full, H = hidden_states.shape
    T = audio_tokens.shape[1]
    NH = 16
    n_cores = 8
    halves = n_cores // B
    Sc = S_full // halves
    rs = float(np.clip(np.float32(residual_scale), 0.0, RES_SCALE_MAX))
    use_qkv_bias = any(np.any(np.asarray(x) != 0) for x in (bq, bk, bv, bo))
    use_gamma_beta = (np.any(np.asarray(ln_gamma) != 1.0)
                      or np.any(np.asarray(ln_beta) != 0.0))

    nc = get_nc(Sc, T, H, NH, rs, use_qkv_bias, use_gamma_beta)
    in_maps = prepare_in_maps(hidden_states, audio_tokens, attention_mask,
                              Wq, bq, Wk, bk, Wv, bv, Wo, bo,
                              use_qkv_bias, n_cores)
    if use_gamma_beta:
        for im in in_maps:
            im["gamma"] = np.asarray(ln_gamma,
                                     np.float32).astype(ml_dtypes.bfloat16)
            im["beta"] = np.asarray(ln_beta,
                                    np.float32).astype(ml_dtypes.bfloat16)

    res = run_bass_kernel_spmd(nc, in_maps, core_ids=list(range(n_cores)))
    out = np.empty((B, S_full, H), np.float32)
    for i in range(n_cores):
        b, half = i // halves, i % halves
        out[b, half * Sc:(half + 1) * Sc, :] = (
            res.results[i]["out"].astype(np.float32))
    return out
